# revision 1
# baseline (speedup 1.0000x reference)
"""Expert-parallel MoE SwiGLU kernel for Trainium2 (8 NeuronCores).

Problem (dense-equivalent reference):
    logits = x @ W_probe.T + b_probe            [T, E]
    scale  = sigmoid(logits) * (logits > tau)   tau from depth-threshold scalars
    per expert e: h = (x@W_up[e].T) * silu(x@W_gate[e].T); down = h@W_down[e].T
    out = sum_e down_e * scale[:, e]

Strategy: expert-parallel (core e owns expert e) + sparse token dispatch.
Routing (tiny probe matmul) runs on host in float64; each core receives only
the tokens active for its expert (padded to a static NP), computes the SwiGLU
FFN for them with bf16 matmuls (fp32 PSUM), applies the combine scale on-device, and the
host scatter-adds the per-expert partial outputs into the full [T, D] result.
Matmuls run in bf16 (fp32 PSUM accumulation, ~4e-3 end-to-end rel err).

Self-contained: hardcodes shapes for T=4096, D=1024, DFF=2048, E=8.
"""

import math

import numpy as np

import concourse.bass as bass  # noqa: F401  (AP types come via tile/bacc)
import concourse.mybir as mybir
import concourse.tile as tile
from concourse import bacc
from concourse._compat import axon_active

T, D, DFF, E = 4096, 1024, 2048, 8
DEPTH_RATIO = 2.0 / 4.0
N_CORES = 8

NP = 1792          # static padded token count per core per batch (max seed-0
                   # expert load is 1770; overflow falls back to extra batches)
TC = 896           # token chunk resident in SBUF
KD = D // 128      # 8  contraction tiles for up/gate
KF = DFF // 128    # 16 contraction tiles for down
F32 = mybir.dt.float32
F32R = mybir.dt.float32r
BF16 = mybir.dt.bfloat16
USE_BF16 = True    # bf16 hides LDWEIGHTS (FWL): ~2x faster than f32r;
                   # end-to-end rel err ~4e-3 vs fp32 reference


def build_nc(np_tok=NP, tc=TC, repeat=1, use_bf16=USE_BF16):
    """Per-core Bass kernel: SwiGLU FFN for one expert over np_tok tokens.

    repeat>1 re-emits the whole computation (timing harness use only): the
    wall-clock slope between repeat values isolates on-device time from
    per-call dispatch overhead.
    """
    assert np_tok % tc == 0 and tc % 128 == 0
    n_a = tc // -(-tc // 512)      # largest even split of tc that is <= 512
    assert tc % n_a == 0 and n_a >= 256
    DT = BF16 if use_bf16 else F32R
    nc = bacc.Bacc(
        "TRN2", target_bir_lowering=False, debug=False, enable_partition_id=False
    )

    xT = nc.dram_tensor("xT", [D, np_tok], DT, kind="ExternalInput").ap()
    wu = nc.dram_tensor("wu", [KF, 128, KD, 128], DT, kind="ExternalInput").ap()
    wg = nc.dram_tensor("wg", [KF, 128, KD, 128], DT, kind="ExternalInput").ap()
    wd = nc.dram_tensor("wd", [DFF, D], DT, kind="ExternalInput").ap()
    sc = nc.dram_tensor("sc", [128, np_tok // 128], F32, kind="ExternalInput").ap()
    out = nc.dram_tensor("out", [np_tok, D], F32, kind="ExternalOutput").ap()

    with tile.TileContext(nc) as tc_ctx:
        with (
            tc_ctx.tile_pool(name="xt", bufs=KD) as xt_pool,
            tc_ctx.tile_pool(name="h", bufs=KF) as h_pool,
            tc_ctx.tile_pool(name="wu", bufs=4) as wu_pool,
            tc_ctx.tile_pool(name="wg", bufs=4) as wg_pool,
            tc_ctx.tile_pool(name="wd", bufs=KF) as wd_pool,
            tc_ctx.tile_pool(name="sil", bufs=2) as sil_pool,
            tc_ctx.tile_pool(name="ob", bufs=3) as ob_pool,
            tc_ctx.tile_pool(name="scp", bufs=1) as sc_pool,
            tc_ctx.tile_pool(name="pU", bufs=2, space="PSUM") as pU,
            tc_ctx.tile_pool(name="pG", bufs=2, space="PSUM") as pG,
            tc_ctx.tile_pool(name="pD", bufs=3, space="PSUM") as pD,
        ):
            sc_sb = sc_pool.tile([128, np_tok // 128], F32)
            nc.sync.dma_start(sc_sb[:], sc[:, :])

            wd_sb = []
            for c in range(repeat * (np_tok // tc)):
                c0 = (c % (np_tok // tc)) * tc
                # activations for this chunk, transposed: [D, tc]
                xt_sb = []
                for kd in range(KD):
                    xtt = xt_pool.tile([128, tc], DT)
                    nc.sync.dma_start(
                        xtt[:], xT[kd * 128:(kd + 1) * 128, c0:c0 + tc]
                    )
                    xt_sb.append(xtt)

                if c == 0:
                    # W_down resident for the whole kernel; emitted after the
                    # prologue-critical xT loads (phase B needs it much later)
                    for kf in range(KF):
                        wdt = wd_pool.tile([128, D], DT)
                        nc.sync.dma_start(wdt[:], wd[kf * 128:(kf + 1) * 128, :])
                        wd_sb.append(wdt)

                # phase A: h[f, t] = up * silu(gate), f on partitions
                h_sb = []
                for ft in range(KF):
                    wut = wu_pool.tile([128, KD, 128], DT)
                    nc.sync.dma_start(wut[:], wu[ft])
                    wgt = wg_pool.tile([128, KD, 128], DT)
                    nc.sync.dma_start(wgt[:], wg[ft])
                    ht = h_pool.tile([128, tc], DT)
                    for t2 in range(tc // n_a):
                        tsl = bass.ts(t2, n_a)
                        pu = pU.tile([128, n_a], F32)
                        pg = pG.tile([128, n_a], F32)
                        for kd in range(KD):
                            nc.tensor.matmul(
                                pu[:], wut[:, kd, :], xt_sb[kd][:, tsl],
                                start=(kd == 0), stop=(kd == KD - 1),
                            )
                        for kd in range(KD):
                            nc.tensor.matmul(
                                pg[:], wgt[:, kd, :], xt_sb[kd][:, tsl],
                                start=(kd == 0), stop=(kd == KD - 1),
                            )
                        sil = sil_pool.tile([128, n_a], F32)
                        nc.scalar.activation(
                            sil[:], pg[:], mybir.ActivationFunctionType.Silu
                        )
                        nc.vector.tensor_mul(ht[:, tsl], pu[:], sil[:])
                    h_sb.append(ht)

                # phase B: down[t, d] = h.T @ wd, then per-token combine scale
                for ts in range(tc // 128):
                    ob = ob_pool.tile([128, D], F32)
                    col = (c % (np_tok // tc)) * (tc // 128) + ts
                    for dt_i in range(D // 512):
                        pd = pD.tile([128, 512], F32)
                        for kf in range(KF):
                            nc.tensor.matmul(
                                pd[:],
                                h_sb[kf][:, ts * 128:(ts + 1) * 128],
                                wd_sb[kf][:, dt_i * 512:(dt_i + 1) * 512],
                                start=(kf == 0), stop=(kf == KF - 1),
                            )
                        nc.vector.tensor_scalar_mul(
                            ob[:, dt_i * 512:(dt_i + 1) * 512],
                            pd[:],
                            sc_sb[:, col:col + 1],
                        )
                    nc.sync.dma_start(
                        out[c0 + ts * 128:c0 + (ts + 1) * 128, :], ob[:]
                    )

    nc.compile()
    return nc


# ---------------------------------------------------------------- host side

def route(x, W_probe, b_probe, tau_base, gamma, w_depth):
    """float64 routing: per-token/expert combine scale + active token ids."""
    x64 = np.asarray(x, np.float64)
    logits = x64 @ np.asarray(W_probe, np.float64).T + np.asarray(b_probe, np.float64)
    arg = float(np.asarray(w_depth).reshape(-1)[0]) * DEPTH_RATIO
    tau = float(np.asarray(tau_base).reshape(-1)[0]) + float(
        np.asarray(gamma).reshape(-1)[0]
    ) * (arg / (1.0 + math.exp(-arg)))
    mask = logits > tau
    scale = np.where(mask, 1.0 / (1.0 + np.exp(-logits)), 0.0)
    ids = [np.nonzero(mask[:, e])[0] for e in range(E)]
    return scale, ids


def _np_dt(use_bf16):
    if use_bf16:
        import ml_dtypes
        return ml_dtypes.bfloat16
    return np.float32


def pack_weights(W_up, W_gate, W_down, use_bf16=USE_BF16):
    """Per-expert DRAM layouts that DMA into SBUF with 4KB/partition runs."""
    dt = _np_dt(use_bf16)
    W_up = np.ascontiguousarray(np.asarray(W_up, np.float32))
    W_gate = np.ascontiguousarray(np.asarray(W_gate, np.float32))
    W_down = np.ascontiguousarray(np.asarray(W_down, np.float32))
    wu_pk, wg_pk, wd_pk = [], [], []
    for e in range(E):
        # [ft, p(d), kd, f] = W[ft*128+f, kd*128+p]
        wu_pk.append(np.ascontiguousarray(
            W_up[e].reshape(KF, 128, KD, 128).transpose(0, 3, 2, 1)).astype(dt))
        wg_pk.append(np.ascontiguousarray(
            W_gate[e].reshape(KF, 128, KD, 128).transpose(0, 3, 2, 1)).astype(dt))
        wd_pk.append(np.ascontiguousarray(W_down[e].T).astype(dt))  # [DFF, D]
    return wu_pk, wg_pk, wd_pk


def make_in_maps(x, scale, ids, wu_pk, wg_pk, wd_pk, batch, np_tok=NP,
                 use_bf16=USE_BF16):
    """Per-core input dicts for one dispatch batch (+ scatter metadata)."""
    x = np.asarray(x, np.float32)
    in_maps, metas = [], []
    for e in range(E):
        sel = ids[e][batch * np_tok:(batch + 1) * np_tok]
        nv = len(sel)
        sel_p = np.zeros(np_tok, np.int64)
        sel_p[:nv] = sel
        xg = x[sel_p]                                   # [np_tok, D]
        xTg = np.ascontiguousarray(xg.T).astype(_np_dt(use_bf16))  # [D, np_tok]
        sc_col = np.zeros(np_tok, np.float32)
        sc_col[:nv] = scale[sel, e]
        sc_pk = np.ascontiguousarray(sc_col.reshape(np_tok // 128, 128).T)
        in_maps.append({
            "xT": xTg, "wu": wu_pk[e], "wg": wg_pk[e],
            "wd": wd_pk[e], "sc": sc_pk,
        })
        metas.append((sel, nv))
    return in_maps, metas


_NC = None
_RUNNER = None


def _get_nc():
    global _NC
    if _NC is None:
        _NC = build_nc()
    return _NC


def _make_pjrt_runner(nc):
    """Cached jitted SPMD executor (axon path), compiled once per process."""
    import jax
    from jax.experimental.shard_map import shard_map
    from jax.sharding import Mesh, PartitionSpec
    from concourse import bass2jax

    bass2jax.install_neuronx_cc_hook()

    in_names, out_names, out_avals, zero_shapes = [], [], [], []
    for alloc in nc.m.functions[0].allocations:
        if not isinstance(alloc, mybir.MemoryLocationSet):
            continue
        name = alloc.memorylocations[0].name
        if alloc.kind == "ExternalInput":
            in_names.append(name)
        elif alloc.kind == "ExternalOutput":
            out_names.append(name)
            shape = tuple(alloc.tensor_shape)
            dtype = mybir.dt.np(alloc.dtype)
            out_avals.append(jax.core.ShapedArray(shape, dtype))
            zero_shapes.append((shape, dtype))
    n_params = len(in_names)
    all_names = in_names + out_names
    donate = tuple(range(n_params, n_params + len(out_names)))

    def _body(*args):
        outs = bass2jax._bass_exec_p.bind(
            *args,
            out_avals=tuple(out_avals),
            in_names=tuple(all_names),
            out_names=tuple(out_names),
            lowering_input_output_aliases=(),
            sim_require_finite=True,
            sim_require_nnan=True,
            nc=nc,
        )
        return tuple(outs)

    devices = jax.devices()[:N_CORES]
    mesh = Mesh(np.asarray(devices), ("core",))
    n_args = n_params + len(out_names)
    sharded = jax.jit(
        shard_map(
            _body,
            mesh=mesh,
            in_specs=(PartitionSpec("core"),) * n_args,
            out_specs=(PartitionSpec("core"),) * len(out_names),
            check_rep=False,
        ),
        donate_argnums=donate,
        keep_unused=True,
    )

    def run(in_maps):
        concat_in = [
            np.concatenate([np.asarray(m[name]) for m in in_maps], axis=0)
            for name in in_names
        ]
        concat_zeros = [
            np.zeros((N_CORES * s[0], *s[1:]), dt) for s, dt in zero_shapes
        ]
        out_arrs = sharded(*concat_in, *concat_zeros)
        return [
            {
                name: np.asarray(out_arrs[i]).reshape(
                    N_CORES, *out_avals[i].shape
                )[c]
                for i, name in enumerate(out_names)
            }
            for c in range(N_CORES)
        ]

    return run


def _get_runner():
    global _RUNNER
    if _RUNNER is None:
        nc = _get_nc()
        if axon_active():
            _RUNNER = _make_pjrt_runner(nc)
        else:
            from concourse.bass_utils import run_bass_kernel_spmd

            def run(in_maps):
                return run_bass_kernel_spmd(
                    nc, in_maps, core_ids=list(range(N_CORES))
                ).results

            _RUNNER = run
    return _RUNNER


def _run_with_retry(in_maps, attempts=4):
    """First execution of a freshly-loaded NEFF is flaky on this stack
    (~50% NRT_EXEC_UNIT_UNRECOVERABLE); reset the jax backend and retry."""
    global _RUNNER
    import time as _time

    for attempt in range(attempts):
        try:
            return _get_runner()(in_maps)
        except Exception:
            if attempt == attempts - 1:
                raise
            _RUNNER = None
            try:
                import jax
                import jax._src.xla_bridge as _xb

                jax.clear_caches()
                _xb._clear_backends()
            except Exception:
                pass
            _time.sleep(3.0 * (attempt + 1))


def kernel(x, W_probe, b_probe, W_up, W_gate, W_down, tau_base, gamma, w_depth):
    x = np.asarray(x, np.float32)
    scale, ids = route(x, W_probe, b_probe, tau_base, gamma, w_depth)
    wu_pk, wg_pk, wd_pk = pack_weights(W_up, W_gate, W_down)
    n_batches = max(1, -(-max(len(i) for i in ids) // NP))
    out = np.zeros((T, D), np.float32)
    for b in range(n_batches):
        in_maps, metas = make_in_maps(x, scale, ids, wu_pk, wg_pk, wd_pk, b)
        results = _run_with_retry(in_maps)
        for e in range(E):
            sel, nv = metas[e]
            if nv:
                out[sel] += results[e]["out"][:nv]
    return out

